# revision 2
# baseline (speedup 1.0000x reference)
"""Multi-head attention (B=2, S=2048, H=16, DH=64, D=1024) on 8 TRN2 NeuronCores.

Sharding: batch x head-group. Core c handles batch b = c//4, head group
hg = c%4 (4 heads = 256 hidden columns). Each core computes its head group's
attention and a partial (row-sliced) output projection; the host sums the 4
partials per batch and adds the bias terms.

Device-side dataflow (per core), everything fp32r (TF32-like) matmuls with
fp32 PSUM accumulation:
  - hsT [D, S] arrives pre-transposed from host.
  - qT = Wq_hg.T-contract: lhsT=Wq slices, rhs=hsT -> qT [256, S] (+bq via
    K=1 rank-1 matmul with a ones row; same for kT).
  - v = hs @ Wv_hg in natural [S, 256] layout (lhsT=hsT slices), stored with
    a ones column per head (v_aug [S, 4x65]) so the PV matmul also produces
    the softmax denominator l as row 64 of the output.
  - Per head: scoresT [sk, sq] = kT.T-contract (lhsT=kT slice, K=64);
    exp(scale*x) on ScalarE -> expT (softmax without max-subtraction: scores
    are bounded by ~+-5 here, exact in fp32); PV: lhsT=v_aug slice,
    rhs=expT -> ctxT_aug [65, sq] accumulated over sk.
  - l -> 1/l (DVE reciprocal), broadcast across 64 partitions via a K=1
    matmul with a ones column, ctxT = ctxT_unnorm * bcast(1/l).
  - outT_partial [D, S] = Wo_hg-contract (lhsT=Wo slices, rhs=ctxT).
Host: out[b] = sum_hg(outT_partial).T + (bo + bv @ Wo)   (bv folds through
the probs@V linearity: probs rows sum to 1 after normalization).
"""

import numpy as np

H = 16
DH = 64
D = 1024
B = 2
S = 2048
HG = 4            # heads per core
DG = HG * DH      # 256 hidden cols per core
SCALE = DH ** -0.5
N_CORES = 8

_cached_nc = None


def _build_nc():
    import concourse.bass as bass  # noqa: F401
    from concourse import bacc
    import concourse.mybir as mybir
    import concourse.tile as tile

    F32 = mybir.dt.float32
    F32R = mybir.dt.float32r
    AFT = mybir.ActivationFunctionType

    nc = bacc.Bacc("TRN2", target_bir_lowering=False)

    hsT = nc.dram_tensor("hsT", [D, S], F32R, kind="ExternalInput")
    wq = nc.dram_tensor("wq", [D, DG], F32R, kind="ExternalInput")
    wk = nc.dram_tensor("wk", [D, DG], F32R, kind="ExternalInput")
    wv = nc.dram_tensor("wv", [D, DG], F32R, kind="ExternalInput")
    wo = nc.dram_tensor("wo", [DG, D], F32R, kind="ExternalInput")
    bq = nc.dram_tensor("bq", [1, DG], F32R, kind="ExternalInput")
    bk = nc.dram_tensor("bk", [1, DG], F32R, kind="ExternalInput")
    outT = nc.dram_tensor("outT", [D, S], F32, kind="ExternalOutput")

    KC = D // 128     # 8 contraction chunks for projections
    SQC = S // 512    # 4 sq chunks of 512
    SKC = S // 128    # 16 sk chunks of 128

    with tile.TileContext(nc) as tc:
        with tc.tile_pool(name="big", bufs=1) as big, \
             tc.tile_pool(name="expp", bufs=2) as expp, \
             tc.tile_pool(name="ep", bufs=2) as ep, \
             tc.tile_pool(name="ost", bufs=3) as ost, \
             tc.tile_pool(name="pbig", bufs=2, space="PSUM") as pbig, \
             tc.tile_pool(name="pctx", bufs=2, space="PSUM") as pctx:

            # ---- persistent SBUF tensors ----
            hsT_sb = big.tile([128, KC, S], F32R)
            wq_sb = big.tile([128, KC, DG], F32R)
            wk_sb = big.tile([128, KC, DG], F32R)
            wv_sb = big.tile([128, KC, DG], F32R)
            wo_sb = big.tile([128, 2, D], F32R)
            bq_sb = big.tile([1, DG], F32R)
            bk_sb = big.tile([1, DG], F32R)
            qT_sb = big.tile([128, 2, S], F32R)
            kT_sb = big.tile([128, 2, S], F32R)
            v_sb = big.tile([128, SKC, HG, 65], F32R)
            ctxT_sb = big.tile([128, 2, S], F32R)
            ones_f = big.tile([65, 512], F32)
            ones_r = big.tile([65, 512], F32R)
            vones_f = big.tile([128, SKC, HG, 1], F32)

            # ---- input DMAs ----
            for kc in range(KC):
                for sqc in range(SQC):
                    nc.sync.dma_start(
                        out=hsT_sb[:, kc, sqc * 512:(sqc + 1) * 512],
                        in_=hsT[kc * 128:(kc + 1) * 128, sqc * 512:(sqc + 1) * 512],
                    )
            nc.sync.dma_start(out=wq_sb, in_=wq[:, :].rearrange("(kc p) n -> p kc n", p=128))
            nc.sync.dma_start(out=wk_sb, in_=wk[:, :].rearrange("(kc p) n -> p kc n", p=128))
            nc.sync.dma_start(out=wv_sb, in_=wv[:, :].rearrange("(kc p) n -> p kc n", p=128))
            nc.sync.dma_start(out=bq_sb, in_=bq[:, :])
            nc.sync.dma_start(out=bk_sb, in_=bk[:, :])
            nc.sync.dma_start(out=wo_sb, in_=wo[:, :].rearrange("(kc p) n -> p kc n", p=128))

            # ---- constants ----
            nc.vector.memset(ones_f, 1.0)
            nc.vector.tensor_copy(ones_r, ones_f)
            nc.vector.memset(vones_f, 1.0)
            nc.vector.tensor_copy(v_sb[:, :, :, 64:65], vones_f)

            # ---- q/k projections (transposed layout) ----
            # qT[d, s] = sum_h Wq[h, d] * hsT[h, s] + bq[d]
            for sqc in range(SQC):
                ssl = slice(sqc * 512, (sqc + 1) * 512)
                for md in range(2):
                    msl = slice(md * 128, (md + 1) * 128)
                    for (w_sb, b_sb, dst) in ((wq_sb, bq_sb, qT_sb), (wk_sb, bk_sb, kT_sb)):
                        ps = pbig.tile([128, 512], mybir.dt.float32, tag="st")
                        for kc in range(KC):
                            nc.tensor.matmul(
                                ps, w_sb[:, kc, msl], hsT_sb[:, kc, ssl],
                                start=(kc == 0), stop=False,
                            )
                        nc.tensor.matmul(
                            ps, b_sb[0:1, msl], ones_r[0:1, 0:512],
                            start=False, stop=True,
                        )
                        nc.vector.tensor_copy(dst[:, md, ssl], ps)

            # ---- v projection (natural layout, per-head ones column) ----
            # v[s, d] = sum_h hs[s, h] * Wv[h, d]   (bv folded on host)
            for skc in range(SKC):
                ksl = slice(skc * 128, (skc + 1) * 128)
                psv = pbig.tile([128, DG], mybir.dt.float32, tag="st")
                for kc in range(KC):
                    nc.tensor.matmul(
                        psv, hsT_sb[:, kc, ksl], wv_sb[:, kc, :],
                        start=(kc == 0), stop=(kc == KC - 1),
                    )
                nc.vector.tensor_copy(
                    v_sb[:, skc, :, 0:64],
                    psv.rearrange("p (h d) -> p h d", h=HG),
                )

            # ---- attention per head ----
            for h in range(HG):
                poff = (h % 2) * 64
                cpart = h // 2
                kT_h = kT_sb[poff:poff + 64, cpart, :]
                qT_h = qT_sb[poff:poff + 64, cpart, :]
                for half in range(2):
                    ctx0 = pctx.tile([65, 512], mybir.dt.float32, tag="ctx")
                    ctx1 = pctx.tile([65, 512], mybir.dt.float32, tag="ctx")
                    ctxs = (ctx0, ctx1)
                    sq0 = half * 1024
                    prev = None  # (expT tile, skc)
                    for skc in range(SKC):
                        ksl = slice(skc * 128, (skc + 1) * 128)
                        sT = pbig.tile([128, 1024], mybir.dt.float32, tag="st")
                        for j in range(2):
                            nc.tensor.matmul(
                                sT[:, j * 512:(j + 1) * 512],
                                kT_h[:, ksl],
                                qT_h[:, sq0 + j * 512: sq0 + (j + 1) * 512],
                                start=True, stop=True,
                            )
                        expT = expp.tile([128, 1024], F32R)
                        nc.scalar.activation(expT, sT, AFT.Exp, scale=SCALE)
                        if prev is not None:
                            pexp, pskc = prev
                            pksl = slice(pskc * 128, (pskc + 1) * 128)
                            for j in range(2):
                                nc.tensor.matmul(
                                    ctxs[j], v_sb[:, pskc, h, 0:65],
                                    pexp[:, j * 512:(j + 1) * 512],
                                    start=(pskc == 0), stop=False,
                                )
                        prev = (expT, skc)
                    pexp, pskc = prev
                    for j in range(2):
                        nc.tensor.matmul(
                            ctxs[j], v_sb[:, pskc, h, 0:65],
                            pexp[:, j * 512:(j + 1) * 512],
                            start=False, stop=True,
                        )
                    # epilogue: normalize by l (row 64) and store ctxT
                    for j in range(2):
                        sqc = half * 2 + j
                        ssl = slice(sqc * 512, (sqc + 1) * 512)
                        invl_f = ep.tile([65, 512], mybir.dt.float32, tag="invf")
                        nc.vector.reciprocal(invl_f[64:65, :], ctxs[j][64:65, :])
                        invl_r = ep.tile([65, 512], F32R, tag="invr")
                        nc.vector.tensor_copy(invl_r[64:65, :], invl_f[64:65, :])
                        psb = pbig.tile([64, 512], mybir.dt.float32, tag="st")
                        nc.tensor.matmul(
                            psb, ones_r[64:65, 0:64], invl_r[64:65, :],
                            start=True, stop=True,
                        )
                        bc = ep.tile([64, 512], mybir.dt.float32, tag="bc")
                        nc.vector.tensor_copy(bc, psb)
                        nc.vector.tensor_mul(
                            ctxT_sb[poff:poff + 64, cpart, ssl],
                            ctxs[j][0:64, :], bc,
                        )

            # ---- output projection (transposed partial output) ----
            # outT[o, s] = sum_d Wo[d, o] * ctxT[d, s]
            for mo in range(8):
                osl = slice(mo * 128, (mo + 1) * 128)
                for sqc in range(SQC):
                    ssl = slice(sqc * 512, (sqc + 1) * 512)
                    pso = pbig.tile([128, 512], mybir.dt.float32, tag="st")
                    for kc2 in range(2):
                        nc.tensor.matmul(
                            pso, wo_sb[:, kc2, osl], ctxT_sb[:, kc2, ssl],
                            start=(kc2 == 0), stop=(kc2 == 1),
                        )
                    ot = ost.tile([128, 512], mybir.dt.float32)
                    nc.scalar.copy(out=ot, in_=pso)
                    nc.sync.dma_start(out=outT[osl, ssl], in_=ot)

    nc.compile()
    return nc


def _get_nc():
    global _cached_nc
    if _cached_nc is None:
        _cached_nc = _build_nc()
    return _cached_nc


def kernel(hidden_states, Wq, bq, Wk, bk, Wv, bv, Wo, bo, _want_trace=False):
    from concourse.bass_utils import run_bass_kernel_spmd

    hidden_states = np.asarray(hidden_states, dtype=np.float32)
    Wq = np.asarray(Wq, dtype=np.float32)
    Wk = np.asarray(Wk, dtype=np.float32)
    Wv = np.asarray(Wv, dtype=np.float32)
    Wo = np.asarray(Wo, dtype=np.float32)
    bq = np.asarray(bq, dtype=np.float32)
    bk = np.asarray(bk, dtype=np.float32)
    bv = np.asarray(bv, dtype=np.float32)
    bo = np.asarray(bo, dtype=np.float32)

    nc = _get_nc()

    hsTs = [np.ascontiguousarray(hidden_states[b].T) for b in range(B)]
    in_maps = []
    for c in range(N_CORES):
        b, hg = divmod(c, HG)
        sl = slice(hg * DG, (hg + 1) * DG)
        in_maps.append({
            "hsT": hsTs[b],
            "wq": np.ascontiguousarray(Wq[:, sl]),
            "wk": np.ascontiguousarray(Wk[:, sl]),
            "wv": np.ascontiguousarray(Wv[:, sl]),
            "wo": np.ascontiguousarray(Wo[sl, :]),
            "bq": np.ascontiguousarray(bq[sl].reshape(1, DG)),
            "bk": np.ascontiguousarray(bk[sl].reshape(1, DG)),
        })

    try:
        res = run_bass_kernel_spmd(
            nc, in_maps, core_ids=list(range(N_CORES)), trace=_want_trace,
        )
    except ModuleNotFoundError:
        res = run_bass_kernel_spmd(
            nc, in_maps, core_ids=list(range(N_CORES)), trace=False,
        )

    bias_full = bo + bv @ Wo  # [D]
    out = np.empty((B, S, D), dtype=np.float32)
    for b in range(B):
        acc = res.results[HG * b]["outT"].astype(np.float64)
        for g in range(1, HG):
            acc = acc + res.results[HG * b + g]["outT"]
        out[b] = acc.T + bias_full

    if _want_trace:
        return out, res
    return out


# revision 19
# speedup vs baseline: 3.3288x; 3.3288x over previous
"""Multi-head attention (B=2, S=2048, H=16, DH=64, D=1024) on 8 TRN2 NeuronCores.

Sharding: batch x head-group. Core c handles batch b = c//4, head group
hg = c%4 (4 heads = 256 hidden columns). Each core computes its head group's
attention and a partial (row-sliced) output projection; the host sums the 4
partials per batch and adds the bias terms.

Device-side dataflow (per core). Projections run as fp32r (TF32-like)
matmuls; attention operands (qT/kT/v/expT) are stored bf16 (measured faster
on HW than fp32r; PSUM accumulation is fp32 throughout):
  - hsT [D, S] arrives pre-transposed from host (fp32r).
  - qT = Wq_hg-contract: lhsT=Wq slices, rhs=hsT -> qT [256, S]; bias bq is
    added per-partition by the ScalarE Identity activation that copies
    PSUM -> SBUF (same for kT).
  - v = hs @ Wv_hg in natural [S, 256] layout (lhsT=hsT slices), stored with
    a ones column per head (v_aug [S, 4x65]) so the PV matmul also produces
    the softmax denominator l as row 64 of the output.
  - Per head pair: the two K=64 scoresT [sk, sq] matmuls (lhsT=kT head
    slices at partition bases 0/64) land in disjoint PE row groups and the
    two banks of one [128, 1024] psum tile, so the PE overlaps them; one
    exp(scale*x) on ScalarE covers both -> expT bf16 (softmax without
    max-subtraction: scores here are bounded by ~+-5, exact in fp32);
    PV: lhsT=v_aug slice, rhs=expT -> ctxT_aug [65, sq] accumulated over sk.
  - l -> 1/l (DVE reciprocal), broadcast across 64 partitions via a K=1
    matmul with a ones column, ctxT = ctxT_unnorm * bcast(1/l)  (fp32r).
  - outT_partial [D, S] = Wo_hg-contract (lhsT=Wo slices, rhs=ctxT, fp32r).
The whole program is software-pipelined: the v projection streams under the
first attention unit, kT/qT for later sq blocks, per-head epilogues, and the
output projection are "dripped" into subsequent attention units' loops.
Host: out[b] = sum_hg(outT_partial).T + (bo + bv @ Wo)   (bv folds through
the probs@V linearity: probs rows sum to 1 after normalization).
"""

import numpy as np

H = 16
DH = 64
D = 1024
B = 2
S = 2048
HG = 4            # heads per core
DG = HG * DH      # 256 hidden cols per core
SCALE = DH ** -0.5
N_CORES = 8

_cached_nc = None


def _build_nc(reps=1):
    import concourse.bass as bass  # noqa: F401
    from concourse import bacc
    import concourse.mybir as mybir
    import concourse.tile as tile

    F32 = mybir.dt.float32
    F32R = mybir.dt.float32r
    BF16 = mybir.dt.bfloat16
    AFT = mybir.ActivationFunctionType

    nc = bacc.Bacc("TRN2", target_bir_lowering=False)

    hsT = nc.dram_tensor("hsT", [D, S], F32R, kind="ExternalInput")
    wq = nc.dram_tensor("wq", [D, DG], F32R, kind="ExternalInput")
    wk = nc.dram_tensor("wk", [D, DG], F32R, kind="ExternalInput")
    wv = nc.dram_tensor("wv", [D, DG], F32R, kind="ExternalInput")
    wo = nc.dram_tensor("wo", [DG, D], F32R, kind="ExternalInput")
    bq = nc.dram_tensor("bq", [2, 128], mybir.dt.float32, kind="ExternalInput")
    bk = nc.dram_tensor("bk", [2, 128], mybir.dt.float32, kind="ExternalInput")
    outT = nc.dram_tensor("outT", [D, S], F32, kind="ExternalOutput")

    KC = D // 128     # 8 contraction chunks for projections
    SQC = S // 512    # 4 sq chunks of 512
    SKC = S // 128    # 16 sk chunks of 128

    with tile.TileContext(nc) as tc:
        with tc.tile_pool(name="big", bufs=1) as big, \
             tc.tile_pool(name="expp", bufs=3) as expp, \
             tc.tile_pool(name="ep", bufs=3) as ep, \
             tc.tile_pool(name="ost", bufs=6) as ost, \
             tc.tile_pool(name="pbig", bufs=2, space="PSUM") as pbig, \
             tc.tile_pool(name="pctx", bufs=4, space="PSUM") as pctx:

            def emit_body():
                # ---- persistent SBUF tensors ----
                hsT_sb = big.tile([128, KC, S], F32R)
                wq_sb = big.tile([128, KC, DG], F32R)
                wk_sb = big.tile([128, KC, DG], F32R)
                wv_sb = big.tile([128, KC, DG], F32R)
                wo_sb = big.tile([128, 2, D], F32R)
                bq_sb = big.tile([128, 2], mybir.dt.float32)
                bk_sb = big.tile([128, 2], mybir.dt.float32)
                qT_sb = big.tile([128, 2, S], BF16)
                kT_sb = big.tile([128, 2, S], BF16)
                v_sb = big.tile([128, SKC, HG, 65], BF16)
                ctxT_sb = big.tile([128, 2, S], F32R)
                ones_f = big.tile([65, 64], F32)
                ones_r = big.tile([65, 64], F32R)
                vones_f = big.tile([128, SKC, HG, 1], F32)

                # ---- input DMAs (ordered by first use: weights gating QKV first,
                # then hsT in sq-column blocks, wv before the fused v-projection,
                # wo last) ----
                wk_r = wk[:, :].rearrange("(kc p) n -> p kc n", p=128)
                wq_r = wq[:, :].rearrange("(kc p) n -> p kc n", p=128)
                wv_r = wv[:, :].rearrange("(kc p) n -> p kc n", p=128)
                nc.sync.dma_start(out=bk_sb, in_=bk[:, :].rearrange("md p -> p md"))
                nc.sync.dma_start(out=bq_sb, in_=bq[:, :].rearrange("md p -> p md"))
                for kc in range(KC):
                    if kc % 2 == 0:
                        nc.sync.dma_start(out=wk_sb[:, kc:kc + 2, :], in_=wk_r[:, kc:kc + 2, :])
                    nc.sync.dma_start(
                        out=hsT_sb[:, kc, 0:512],
                        in_=hsT[kc * 128:(kc + 1) * 128, 0:512],
                    )
                for kc in range(0, KC, 2):
                    nc.sync.dma_start(out=wq_sb[:, kc:kc + 2, :], in_=wq_r[:, kc:kc + 2, :])
                for kc in range(0, KC, 2):
                    nc.sync.dma_start(out=wv_sb[:, kc:kc + 2, :], in_=wv_r[:, kc:kc + 2, :])
                for sqc in range(1, SQC):
                    for kc in range(KC):
                        nc.sync.dma_start(
                            out=hsT_sb[:, kc, sqc * 512:(sqc + 1) * 512],
                            in_=hsT[kc * 128:(kc + 1) * 128, sqc * 512:(sqc + 1) * 512],
                        )
                wo_r = wo[:, :].rearrange("(kc p) n -> p kc n", p=128)
                for oc in range(0, D, 256):
                    nc.sync.dma_start(out=wo_sb[:, :, oc:oc + 256], in_=wo_r[:, :, oc:oc + 256])

                # ---- constants ----
                nc.vector.memset(ones_f, 1.0)
                nc.vector.tensor_copy(ones_r, ones_f)
                nc.vector.memset(vones_f, 1.0)
                nc.vector.tensor_copy(v_sb[:, :, :, 64:65], vones_f)

                # ---- helper emitters ----
                def qk_proj(w_sb, b_sb, dst, sqc, md):
                    ssl = slice(sqc * 512, (sqc + 1) * 512)
                    msl = slice(md * 128, (md + 1) * 128)
                    ps = pbig.tile([128, 512], mybir.dt.float32, tag="st", name="ps_qk")
                    for kc in range(KC):
                        nc.tensor.matmul(
                            ps, w_sb[:, kc, msl], hsT_sb[:, kc, ssl],
                            start=(kc == 0), stop=(kc == KC - 1),
                        )
                    nc.scalar.activation(
                        dst[:, md, ssl], ps, AFT.Identity, bias=b_sb[:, md:md + 1],
                    )

                def v_proj(skc):
                    ksl = slice(skc * 128, (skc + 1) * 128)
                    psv = pbig.tile([128, DG], mybir.dt.float32, tag="st", name="psv")
                    for kc in range(KC):
                        nc.tensor.matmul(
                            psv, hsT_sb[:, kc, ksl], wv_sb[:, kc, :],
                            start=(kc == 0), stop=(kc == KC - 1),
                        )
                    nc.vector.tensor_copy(
                        v_sb[:, skc, :, 0:64],
                        psv.rearrange("p (h d) -> p h d", h=HG),
                    )

                def epilogue(h, ctx, sqc):
                    # normalize ctxT_unnorm (rows 0:64) by l (row 64), write ctxT
                    poff = (h % 2) * 64
                    cpart = h // 2
                    ssl = slice(sqc * 512, (sqc + 1) * 512)
                    invl_r = ep.tile([65, 512], F32R, tag="invr", name="invl_r")
                    with nc.allow_low_precision(reason="1/l rounded to fp32r feeds the fp32r broadcast matmul"):
                        nc.vector.reciprocal(invl_r[64:65, :], ctx[64:65, :])
                    psb = pbig.tile([64, 512], mybir.dt.float32, tag="st", name="psb")
                    nc.tensor.matmul(
                        psb, ones_r[64:65, 0:64], invl_r[64:65, :],
                        start=True, stop=True,
                    )
                    bc = ep.tile([64, 512], mybir.dt.float32, tag="bc", name="bc")
                    nc.vector.tensor_copy(bc, psb)
                    nc.vector.tensor_mul(
                        ctxT_sb[poff:poff + 64, cpart, ssl],
                        ctx[0:64, :], bc,
                    )

                out_copy_toggle = [0]
                in_tail = [False]

                def out_proj(mo, sqc):
                    osl = slice(mo * 128, (mo + 1) * 128)
                    ssl = slice(sqc * 512, (sqc + 1) * 512)
                    pso = pctx.tile([128, 512], mybir.dt.float32, tag="ctx", name="pso")
                    for kc2 in range(2):
                        nc.tensor.matmul(
                            pso, wo_sb[:, kc2, osl], ctxT_sb[:, kc2, ssl],
                            start=(kc2 == 0), stop=(kc2 == 1),
                        )
                    ot = ost.tile([128, 512], mybir.dt.float32, name="ot")
                    if in_tail[0] and out_copy_toggle[0] % 2 == 0:
                        nc.scalar.copy(out=ot, in_=pso)
                    else:
                        nc.vector.tensor_copy(ot, pso)
                    out_copy_toggle[0] += 1
                    nc.sync.dma_start(out=outT[osl, ssl], in_=ot)

                # ---- pre-attention projections: kT sqc 0-1, qT sqc 0 ----
                # (kT sqc 2-3 are dripped into unit 0 before its skc>=8 needs them)
                for sqc in range(2):
                    for md in range(2):
                        qk_proj(wk_sb, bk_sb, kT_sb, sqc, md)
                for md in range(2):
                    qk_proj(wq_sb, bq_sb, qT_sb, 0, md)

                # deferred work, dripped into later attention units' loops.
                # prio_q (qT projections + ctx-psum-releasing epilogues) drains
                # ahead of slack_q and every iteration, so unit u's ctx slots free
                # within unit u+1 and qT(sqc) is ready one sqc ahead.
                from collections import deque
                prio_q = deque()
                slack_q = deque()

                def drip(slack_ok):
                    if prio_q:
                        prio_q.popleft()()
                    elif slack_ok and slack_q:
                        slack_q.popleft()()

                # ---- attention units: (sqc, head-pair), software-pipelined ----
                # Each pair's two K=64 scoresT matmuls go to disjoint PE row
                # groups (partitions 0:64 / 64:128) and the two banks of one
                # [128, 1024] psum tile, so the PE overlaps them and a single exp
                # covers both heads.
                units = [(sqc, pair) for sqc in range(SQC) for pair in range(2)]
                for ui, (sqc, pair) in enumerate(units):
                    ssl = slice(sqc * 512, (sqc + 1) * 512)
                    ctx0 = pctx.tile([65, 512], mybir.dt.float32, tag="ctx", name="ctx0")
                    ctx1 = pctx.tile([65, 512], mybir.dt.float32, tag="ctx", name="ctx1")
                    ctxs = (ctx0, ctx1)
                    if ui == 0:
                        for k_sqc in (2, 3):
                            for md in range(2):
                                prio_q.append(
                                    lambda k_sqc=k_sqc, md=md: qk_proj(wk_sb, bk_sb, kT_sb, k_sqc, md))
                    if ui == 1:
                        # qT for sqc 1..3, one sqc ahead of first use
                        for q_sqc in (1, 2, 3):
                            for md in range(2):
                                prio_q.append(
                                    lambda q_sqc=q_sqc, md=md: qk_proj(wq_sb, bq_sb, qT_sb, q_sqc, md))
                    prev = None
                    for skc in range(SKC):
                        if ui == 0:
                            v_proj(skc)  # stream the v projection under unit 0
                        ksl = slice(skc * 128, (skc + 1) * 128)
                        sT = pbig.tile([128, 1024], mybir.dt.float32, tag="st", name="sT")
                        for hh in range(2):
                            nc.tensor.matmul(
                                sT[:, hh * 512:(hh + 1) * 512],
                                kT_sb[hh * 64:(hh + 1) * 64, pair, ksl],
                                qT_sb[hh * 64:(hh + 1) * 64, pair, ssl],
                                start=True, stop=True,
                            )
                        expT = expp.tile([128, 1024], BF16, name="expT")
                        nc.scalar.activation(expT, sT, AFT.Exp, scale=SCALE)
                        if prev is not None:
                            pexp, pskc = prev
                            for hh in range(2):
                                nc.tensor.matmul(
                                    ctxs[hh], v_sb[:, pskc, pair * 2 + hh, 0:65],
                                    pexp[:, hh * 512:(hh + 1) * 512],
                                    start=(pskc == 0), stop=False,
                                )
                        prev = (expT, skc)
                        if skc >= 2:
                            drip(slack_ok=(skc % 2 == 0))
                    pexp, pskc = prev
                    for hh in range(2):
                        nc.tensor.matmul(
                            ctxs[hh], v_sb[:, pskc, pair * 2 + hh, 0:65],
                            pexp[:, hh * 512:(hh + 1) * 512],
                            start=False, stop=True,
                        )
                    # defer this unit's epilogues into the next unit's loop
                    for hh in range(2):
                        prio_q.append(
                            lambda h=pair * 2 + hh, ctx=ctxs[hh], sqc=sqc: epilogue(h, ctx, sqc))
                    if pair == 1:
                        # both pairs of this sqc done (after epilogues): queue out-proj
                        for mo in range(8):
                            slack_q.append(lambda mo=mo, sqc=sqc: out_proj(mo, sqc))

                # drain remaining deferred work
                in_tail[0] = True
                while prio_q or slack_q:
                    drip(slack_ok=True)

            for _rep in range(reps):
                emit_body()

    nc.compile()
    return nc


def _get_nc(reps=1):
    global _cached_nc
    if reps != 1:
        return _build_nc(reps)
    if _cached_nc is None:
        _cached_nc = _build_nc()
    return _cached_nc


def kernel(hidden_states, Wq, bq, Wk, bk, Wv, bv, Wo, bo, _want_trace=False):
    from concourse.bass_utils import run_bass_kernel_spmd

    hidden_states = np.asarray(hidden_states, dtype=np.float32)
    Wq = np.asarray(Wq, dtype=np.float32)
    Wk = np.asarray(Wk, dtype=np.float32)
    Wv = np.asarray(Wv, dtype=np.float32)
    Wo = np.asarray(Wo, dtype=np.float32)
    bq = np.asarray(bq, dtype=np.float32)
    bk = np.asarray(bk, dtype=np.float32)
    bv = np.asarray(bv, dtype=np.float32)
    bo = np.asarray(bo, dtype=np.float32)

    nc = _get_nc()

    hsTs = [np.ascontiguousarray(hidden_states[b].T) for b in range(B)]
    in_maps = []
    for c in range(N_CORES):
        b, hg = divmod(c, HG)
        sl = slice(hg * DG, (hg + 1) * DG)
        in_maps.append({
            "hsT": hsTs[b],
            "wq": np.ascontiguousarray(Wq[:, sl]),
            "wk": np.ascontiguousarray(Wk[:, sl]),
            "wv": np.ascontiguousarray(Wv[:, sl]),
            "wo": np.ascontiguousarray(Wo[sl, :]),
            "bq": np.ascontiguousarray(bq[sl].reshape(2, 128)),
            "bk": np.ascontiguousarray(bk[sl].reshape(2, 128)),
        })

    try:
        res = run_bass_kernel_spmd(
            nc, in_maps, core_ids=list(range(N_CORES)), trace=_want_trace,
        )
    except ModuleNotFoundError:
        res = run_bass_kernel_spmd(
            nc, in_maps, core_ids=list(range(N_CORES)), trace=False,
        )

    bias_full = bo + bv @ Wo  # [D]
    out = np.empty((B, S, D), dtype=np.float32)
    for b in range(B):
        acc = res.results[HG * b]["outT"].astype(np.float64)
        for g in range(1, HG):
            acc = acc + res.results[HG * b + g]["outT"]
        out[b] = acc.T + bias_full

    if _want_trace:
        return out, res
    return out

